# revision 15
# baseline (speedup 1.0000x reference)
import sys

if "/opt/trn_rl_repo" not in sys.path:
    sys.path.insert(0, "/opt/trn_rl_repo")

import numpy as np

import concourse.bass as bass
import concourse.bacc as bacc
import concourse.mybir as mybir
import concourse.tile as tile
from concourse.bass_utils import run_bass_kernel_spmd

B, P, N, F, L = 4, 64, 48, 32, 3
NCORES = 8
PPC = (B * P) // NCORES          # polylines per core = 32
NBLK = 4                         # column blocks per core
BPP = PPC // NBLK                # polylines per block = 8
D = [32, 64, 128]                # per-layer input feature dim
EPS = np.float32(1e-6)
FP = mybir.dt.float32


def _build_packs(pars):
    """Pack each layer's weights/biases into a [128, ncols] f32 array (lhsT layout).

    Math (per layer, d=in feats, h=2d):
      h_ = relu(x @ we + be)
      node = alpha*(h_ @ wE1) + bcast_poly( sum_j mi_j*h_j @ wE2 + alpha (x) bE )
      x' = relu(node @ w1 + b1) @ w2 + b2
    where alpha=cnt/(cnt+eps), mi=m/(cnt+eps). Weights stored as [d_in, d_out]
    which is exactly matmul lhsT.
    """
    packs, offs = [], []
    for l in range(L):
        d, h = D[l], 2 * D[l]
        kcs = max(1, h // 128)
        mcs = max(1, h // 128)
        we, be, wE, bE, w1, b1, w2, b2 = pars[l]
        ncols = d + 3 * h + 2 * kcs * h + 1 + 2 * mcs
        pk = np.zeros((128, ncols), np.float32)
        co = {}
        c = 0
        co["we"] = c; pk[:d, c:c + d] = we; c += d
        co["wE1"] = c; pk[:d, c:c + h] = wE[:d]; c += h
        co["wE2"] = c; pk[:d, c:c + h] = wE[d:2 * d]; c += h
        co["bE"] = c; pk[0, c:c + h] = bE; c += h
        co["w1"] = c
        for kc in range(kcs):
            ksz = min(128, h - kc * 128)
            pk[:ksz, c:c + h] = w1[kc * 128:kc * 128 + ksz, :]
            c += h
        co["w2"] = c
        for kc in range(kcs):
            ksz = min(128, h - kc * 128)
            pk[:ksz, c:c + h] = w2[kc * 128:kc * 128 + ksz, :]
            c += h
        co["be"] = c; pk[:d, c] = be; c += 1
        co["b1"] = c
        for mc in range(mcs):
            msz = min(128, h - mc * 128)
            pk[:msz, c + mc] = b1[mc * 128:mc * 128 + msz]
        c += mcs
        co["b2"] = c
        for mc in range(mcs):
            msz = min(128, h - mc * 128)
            pk[:msz, c + mc] = b2[mc * 128:mc * 128 + msz]
        c += mcs
        assert c == ncols
        packs.append(pk)
        offs.append(co)
    return packs, offs


def _build_program(pack_ncols):
    nc = bacc.Bacc()
    xin_d = nc.declare_dram_parameter("xin", [F, PPC, N], FP, isOutput=False)
    mi_d = nc.declare_dram_parameter("mi", [128, PPC, N], FP, isOutput=False)
    al_d = nc.declare_dram_parameter("al", [128, PPC, N], FP, isOutput=False)
    av_d = nc.declare_dram_parameter("avec", [1, PPC], FP, isOutput=False)
    i8_d = nc.declare_dram_parameter("i8", [BPP, BPP * N], FP, isOutput=False)
    pk_d = [
        nc.declare_dram_parameter(f"pk{l}", [128, pack_ncols[l]], FP, isOutput=False)
        for l in range(L)
    ]
    out_d = nc.declare_dram_parameter("out", [2, 128, PPC], FP, isOutput=True)

    AF = mybir.ActivationFunctionType
    OP = mybir.AluOpType
    AX = mybir.AxisListType

    with tile.TileContext(nc) as tc:
        with tc.tile_pool(name="sb", bufs=1) as sb, \
             tc.tile_pool(name="ps", bufs=4, space=bass.MemorySpace.PSUM) as ps, \
             tc.tile_pool(name="ps2", bufs=2, space=bass.MemorySpace.PSUM) as ps2:
            xin_t = sb.tile([F, PPC, N], FP, name="xin_t")
            pk_t = [sb.tile([128, pack_ncols[l]], FP, name=f"pk_t{l}") for l in range(L)]
            mi_t = sb.tile([128, PPC, N], FP, name="mi_t")
            al_t = sb.tile([128, PPC, N], FP, name="al_t")
            av_t = sb.tile([1, PPC], FP, name="av_t")
            i8_t = sb.tile([BPP, BPP * N], FP, name="i8_t")

            nc.sync.dma_start(xin_t[:], xin_d[:])
            nc.sync.dma_start(pk_t[0][:], pk_d[0][:])
            nc.sync.dma_start(mi_t[:], mi_d[:])
            nc.sync.dma_start(al_t[:], al_d[:])
            nc.sync.dma_start(av_t[:], av_d[:])
            nc.sync.dma_start(i8_t[:], i8_d[:])
            nc.sync.dma_start(pk_t[1][:], pk_d[1][:])
            nc.sync.dma_start(pk_t[2][:], pk_d[2][:])

            x_cur = [xin_t]
            for l in range(L):
                d, h = D[l], 2 * D[l]
                mcs = max(1, h // 128)
                kcs = mcs
                M = [min(128, h - mc * 128) for mc in range(mcs)]
                co = _OFFS[l]
                pk = pk_t[l]

                h3 = sb.tile([d, PPC, N], FP, name=f"h3_{l}", tag="h3")
                t3 = sb.tile([d, PPC, N], FP, name=f"t3_{l}", tag="t3")
                Ht = sb.tile([d, PPC], FP, name=f"Ht_{l}", tag="Ht")
                s2 = sb.tile([BPP, NBLK * h], FP, name=f"s2_{l}", tag="s2")
                node = [sb.tile([M[mc], PPC, N], FP, name=f"node_{l}_{mc}", tag=f"node{mc}") for mc in range(mcs)]
                u = [sb.tile([M[mc], PPC, N], FP, name=f"u_{l}_{mc}", tag=f"u{mc}") for mc in range(mcs)]
                xn = [sb.tile([M[mc], PPC, N], FP, name=f"xn_{l}_{mc}", tag=f"xn{mc}") for mc in range(mcs)]

                # h3 = relu(we.T @ x + be)
                for b in range(NBLK):
                    ph = ps.tile([d, BPP, N], FP, name=f"ph_{l}_{b}", tag="pmm")
                    nc.tensor.matmul(
                        ph[:],
                        pk[0:d, co["we"]:co["we"] + d],
                        x_cur[0][0:d, b * BPP:(b + 1) * BPP, :],
                        start=True, stop=True,
                    )
                    nc.scalar.activation(
                        h3[:, b * BPP:(b + 1) * BPP, :], ph[:], AF.Relu,
                        bias=pk[0:d, co["be"]:co["be"] + 1],
                    )

                # Ht = sum_n (h3 * mi)  per polyline
                nc.vector.tensor_tensor(t3[:], h3[:], mi_t[0:d, :, :], OP.mult)
                nc.vector.tensor_reduce(Ht[:], t3[:], AX.X, OP.add)

                # s2[0:8, b*h + f] = (Ht.T @ wE2 + alpha (x) bE)[poly b*8.., f]
                # (block-major so matmul lhsT slices start at partition 0)
                for b in range(NBLK):
                    pss = ps2.tile([BPP, h], FP, name=f"pss_{l}_{b}", tag="pss")
                    nc.tensor.matmul(pss[:], Ht[:, b * BPP:(b + 1) * BPP],
                                     pk[0:d, co["wE2"]:co["wE2"] + h],
                                     start=True, stop=False)
                    nc.tensor.matmul(pss[:], av_t[0:1, b * BPP:(b + 1) * BPP],
                                     pk[0:1, co["bE"]:co["bE"] + h],
                                     start=False, stop=True)
                    nc.vector.tensor_copy(s2[:, b * h:(b + 1) * h], pss[:])

                # t3 = h3 * alpha (per column)
                nc.vector.tensor_tensor(t3[:], h3[:], al_t[0:d, :, :], OP.mult)

                # node = wE1.T @ t3 + bcast_poly(s2)
                for b in range(NBLK):
                    for mc in range(mcs):
                        pn = ps.tile([M[mc], BPP, N], FP, name=f"pn_{l}_{b}_{mc}", tag="pmm")
                        nc.tensor.matmul(
                            pn[:],
                            pk[0:d, co["wE1"] + mc * 128:co["wE1"] + mc * 128 + M[mc]],
                            t3[:, b * BPP:(b + 1) * BPP, :],
                            start=True, stop=False,
                        )
                        nc.tensor.matmul(
                            pn[:],
                            s2[:, b * h + mc * 128:b * h + mc * 128 + M[mc]],
                            i8_t[:],
                            start=False, stop=True,
                        )
                        nc.vector.tensor_copy(node[mc][:, b * BPP:(b + 1) * BPP, :], pn[:])

                # u = relu(w1.T @ node + b1)
                for b in range(NBLK):
                    for mc in range(mcs):
                        pu = ps.tile([M[mc], BPP, N], FP, name=f"pu_{l}_{b}_{mc}", tag="pmm")
                        for kc in range(kcs):
                            ksz = min(128, h - kc * 128)
                            nc.tensor.matmul(
                                pu[:],
                                pk[0:ksz, co["w1"] + kc * h + mc * 128:co["w1"] + kc * h + mc * 128 + M[mc]],
                                node[kc][:, b * BPP:(b + 1) * BPP, :],
                                start=(kc == 0), stop=(kc == kcs - 1),
                            )
                        nc.scalar.activation(
                            u[mc][:, b * BPP:(b + 1) * BPP, :], pu[:], AF.Relu,
                            bias=pk[0:M[mc], co["b1"] + mc:co["b1"] + mc + 1],
                        )

                # xn = w2.T @ u + b2
                for b in range(NBLK):
                    for mc in range(mcs):
                        px = ps.tile([M[mc], BPP, N], FP, name=f"px_{l}_{b}_{mc}", tag="pmm")
                        for kc in range(kcs):
                            ksz = min(128, h - kc * 128)
                            nc.tensor.matmul(
                                px[:],
                                pk[0:ksz, co["w2"] + kc * h + mc * 128:co["w2"] + kc * h + mc * 128 + M[mc]],
                                u[kc][:, b * BPP:(b + 1) * BPP, :],
                                start=(kc == 0), stop=(kc == kcs - 1),
                            )
                        nc.vector.tensor_scalar_add(
                            xn[mc][:, b * BPP:(b + 1) * BPP, :], px[:],
                            pk[0:M[mc], co["b2"] + mc:co["b2"] + mc + 1],
                        )
                x_cur = xn

            # final masked mean pool: out[mc] = sum_n (x * mi)
            for mc in range(2):
                xm = sb.tile([128, PPC, N], FP, name=f"xm_{mc}", tag="xm")
                nc.vector.tensor_tensor(xm[:], x_cur[mc][:], mi_t[:], OP.mult)
                Hf = sb.tile([128, PPC], FP, name=f"Hf_{mc}")
                nc.vector.tensor_reduce(Hf[:], xm[:], AX.X, OP.add)
                nc.sync.dma_start(out_d[mc], Hf[:])

    nc.finalize()
    return nc


_OFFS = None


def run(x, masks, params, trace=False):
    global _OFFS
    x = np.asarray(x, dtype=np.float32)
    m = np.asarray(masks)
    pars = [[np.asarray(t, dtype=np.float32) for t in layer] for layer in params]

    packs, offs = _build_packs(pars)
    _OFFS = offs

    i8 = np.zeros((BPP, BPP * N), np.float32)
    for k in range(BPP):
        i8[k, k * N:(k + 1) * N] = 1.0

    xf = x.reshape(B * P, N, F)
    mf = m.reshape(B * P, N).astype(np.float32)

    in_maps = []
    for c in range(NCORES):
        s = slice(c * PPC, (c + 1) * PPC)
        xc = np.ascontiguousarray(xf[s].transpose(2, 0, 1))          # [F, PPC, N]
        mc_ = mf[s]                                                  # [PPC, N]
        cnt = mc_.sum(axis=1, dtype=np.float32)
        inv = np.float32(1.0) / (cnt + EPS)
        alpha = cnt * inv
        mi = np.ascontiguousarray(
            np.broadcast_to((mc_ * inv[:, None])[None], (128, PPC, N))
        )
        al = np.ascontiguousarray(
            np.broadcast_to(alpha[None, :, None], (128, PPC, N))
        )
        im = {
            "xin": xc,
            "mi": mi,
            "al": al,
            "avec": np.ascontiguousarray(alpha.reshape(1, PPC)),
            "i8": i8,
        }
        for l in range(L):
            im[f"pk{l}"] = packs[l]
        in_maps.append(im)

    nc = _build_program([p.shape[1] for p in packs])
    res = run_bass_kernel_spmd(nc, in_maps, list(range(NCORES)), trace=trace)

    outs = []
    for c in range(NCORES):
        o = res.results[c]["out"].reshape(2 * 128, PPC).T             # [PPC, 256]
        outs.append(o)
    full = np.concatenate(outs, axis=0).reshape(B, P, 2 * 128)
    return np.ascontiguousarray(full.astype(np.float32)), res.exec_time_ns


def kernel(**inputs):
    out, _ = run(**inputs)
    return out


# revision 24
# speedup vs baseline: 1.5506x; 1.5506x over previous
import sys

if "/opt/trn_rl_repo" not in sys.path:
    sys.path.insert(0, "/opt/trn_rl_repo")

import numpy as np

import concourse.bass as bass
import concourse.bacc as bacc
import concourse.mybir as mybir
import concourse.tile as tile
from concourse.bass_utils import run_bass_kernel_spmd

B, P, N, F, L = 4, 64, 48, 32, 3
NCORES = 8
PPC = (B * P) // NCORES          # polylines per core = 32
NBLK = 4                         # column blocks per core
BPP = PPC // NBLK                # polylines per block = 8
D = [32, 64, 128]                # per-layer input feature dim
EPS = np.float32(1e-6)
FP = mybir.dt.float32
FR = mybir.dt.float32r


def _build_packs(pars):
    """Pack each layer's weights/biases into a [128, ncols] f32 array (lhsT layout).

    Math (per layer, d=in feats, h=2d):
      h_ = relu(x @ we + be)
      node = alpha*(h_ @ wE1) + bcast_poly( sum_j mi_j*h_j @ wE2 + alpha (x) bE )
      x' = relu(node @ w1 + b1) @ w2 + b2
    where alpha=cnt/(cnt+eps), mi=m/(cnt+eps). Weights stored as [d_in, d_out]
    which is exactly matmul lhsT.
    """
    packs, offs = [], []
    for l in range(L):
        d, h = D[l], 2 * D[l]
        kcs = max(1, h // 128)
        mcs = max(1, h // 128)
        we, be, wE, bE, w1, b1, w2, b2 = pars[l]
        ncols = d + 3 * h + 2 * kcs * h + 1 + 2 * mcs
        pk = np.zeros((128, ncols), np.float32)
        co = {}
        c = 0
        co["we"] = c; pk[:d, c:c + d] = we; c += d
        co["wE1"] = c; pk[:d, c:c + h] = wE[:d]; c += h
        co["wE2"] = c; pk[:d, c:c + h] = wE[d:2 * d]; c += h
        co["bE"] = c; pk[0, c:c + h] = bE; c += h
        co["w1"] = c
        for kc in range(kcs):
            ksz = min(128, h - kc * 128)
            pk[:ksz, c:c + h] = w1[kc * 128:kc * 128 + ksz, :]
            c += h
        co["w2"] = c
        for kc in range(kcs):
            ksz = min(128, h - kc * 128)
            pk[:ksz, c:c + h] = w2[kc * 128:kc * 128 + ksz, :]
            c += h
        co["be"] = c; pk[:d, c] = be; c += 1
        co["b1"] = c
        for mc in range(mcs):
            msz = min(128, h - mc * 128)
            pk[:msz, c + mc] = b1[mc * 128:mc * 128 + msz]
        c += mcs
        co["b2"] = c
        for mc in range(mcs):
            msz = min(128, h - mc * 128)
            pk[:msz, c + mc] = b2[mc * 128:mc * 128 + msz]
        c += mcs
        assert c == ncols
        packs.append(pk)
        offs.append(co)
    return packs, offs


def _build_program(pack_ncols):
    nc = bacc.Bacc()
    xin_d = nc.declare_dram_parameter("xin", [F, PPC, N], FP, isOutput=False)
    mi_d = nc.declare_dram_parameter("mi", [128, PPC, N], FP, isOutput=False)
    al_d = nc.declare_dram_parameter("al", [128, PPC, N], FP, isOutput=False)
    av_d = nc.declare_dram_parameter("avec", [1, PPC], FP, isOutput=False)
    i8_d = nc.declare_dram_parameter("i8", [BPP, BPP * N], FP, isOutput=False)
    pk_d = [
        nc.declare_dram_parameter(f"pk{l}", [128, pack_ncols[l]], FP, isOutput=False)
        for l in range(L)
    ]
    out_d = nc.declare_dram_parameter("out", [2, 128, PPC], FP, isOutput=True)

    AF = mybir.ActivationFunctionType
    OP = mybir.AluOpType
    AX = mybir.AxisListType

    with tile.TileContext(nc) as tc:
        with tc.tile_pool(name="sb", bufs=1) as sb, \
             tc.tile_pool(name="ps", bufs=4, space=bass.MemorySpace.PSUM) as ps, \
             tc.tile_pool(name="ps2", bufs=2, space=bass.MemorySpace.PSUM) as ps2:
            xin_t = sb.tile([F, PPC, N], FR, name="xin_t")
            pk_t = [sb.tile([128, pack_ncols[l]], FR, name=f"pk_t{l}") for l in range(L)]
            mi_t = sb.tile([128, PPC, N], FP, name="mi_t")
            al_t = sb.tile([128, PPC, N], FP, name="al_t")
            av_t = sb.tile([1, PPC], FR, name="av_t")
            i8_t = sb.tile([BPP, BPP * N], FR, name="i8_t")

            nc.sync.dma_start(xin_t[:], xin_d[:].bitcast(FR))
            nc.sync.dma_start(pk_t[0][:], pk_d[0][:].bitcast(FR))
            nc.sync.dma_start(mi_t[:], mi_d[:])
            nc.sync.dma_start(al_t[:], al_d[:])
            nc.sync.dma_start(av_t[:], av_d[:].bitcast(FR))
            nc.sync.dma_start(i8_t[:], i8_d[:].bitcast(FR))
            nc.sync.dma_start(pk_t[1][:], pk_d[1][:].bitcast(FR))
            nc.sync.dma_start(pk_t[2][:], pk_d[2][:].bitcast(FR))

            x_cur = [xin_t]
            for l in range(L):
                d, h = D[l], 2 * D[l]
                mcs = max(1, h // 128)
                kcs = mcs
                M = [min(128, h - mc * 128) for mc in range(mcs)]
                co = _OFFS[l]
                pk = pk_t[l]

                h3 = sb.tile([d, PPC, N], FP, name=f"h3_{l}", tag="h3")
                t3 = sb.tile([d, PPC, N], FR, name=f"t3_{l}", tag="t3")
                Ht = sb.tile([d, PPC], FR, name=f"Ht_{l}", tag="Ht")
                s2 = sb.tile([BPP, NBLK * h], FR, name=f"s2_{l}", tag="s2")
                node = [sb.tile([M[mc], PPC, N], FR, name=f"node_{l}_{mc}", tag=f"node{mc}") for mc in range(mcs)]
                u = [sb.tile([M[mc], PPC, N], FR, name=f"u_{l}_{mc}", tag=f"u{mc}") for mc in range(mcs)]
                xn = [sb.tile([M[mc], PPC, N], FR, name=f"xn_{l}_{mc}", tag=f"xn{mc}") for mc in range(mcs)]

                # h3 = relu(we.T @ x + be)
                for b in range(NBLK):
                    ph = ps.tile([d, BPP, N], FP, name=f"ph_{l}_{b}", tag="pmm")
                    nc.tensor.matmul(
                        ph[:],
                        pk[0:d, co["we"]:co["we"] + d],
                        x_cur[0][0:d, b * BPP:(b + 1) * BPP, :],
                        start=True, stop=True,
                    )
                    nc.scalar.activation(
                        h3[:, b * BPP:(b + 1) * BPP, :], ph[:], AF.Relu,
                        bias=pk[0:d, co["be"]:co["be"] + 1].bitcast(FP),
                    )

                # Ht = sum_n (h3 * mi)  per polyline
                nc.vector.tensor_tensor(t3[:], h3[:], mi_t[0:d, :, :], OP.mult)
                with nc.allow_low_precision(reason="fp32r output, fp32 accumulation"):
                    nc.vector.tensor_reduce(Ht[:], t3[:].bitcast(FP), AX.X, OP.add)

                # s2[0:8, b*h + f] = (Ht.T @ wE2 + alpha (x) bE)[poly b*8.., f]
                # (block-major so matmul lhsT slices start at partition 0)
                for b in range(NBLK):
                    pss = ps2.tile([BPP, h], FP, name=f"pss_{l}_{b}", tag="pss")
                    nc.tensor.matmul(pss[:], Ht[:, b * BPP:(b + 1) * BPP],
                                     pk[0:d, co["wE2"]:co["wE2"] + h],
                                     start=True, stop=False)
                    nc.tensor.matmul(pss[:], av_t[0:1, b * BPP:(b + 1) * BPP],
                                     pk[0:1, co["bE"]:co["bE"] + h],
                                     start=False, stop=True)
                    nc.vector.tensor_copy(s2[:, b * h:(b + 1) * h], pss[:])

                # t3 = h3 * alpha (per column)
                nc.vector.tensor_tensor(t3[:], h3[:], al_t[0:d, :, :], OP.mult)

                # node = wE1.T @ t3 + bcast_poly(s2)
                for b in range(NBLK):
                    for mc in range(mcs):
                        pn = ps.tile([M[mc], BPP, N], FP, name=f"pn_{l}_{b}_{mc}", tag="pmm")
                        nc.tensor.matmul(
                            pn[:],
                            pk[0:d, co["wE1"] + mc * 128:co["wE1"] + mc * 128 + M[mc]],
                            t3[:, b * BPP:(b + 1) * BPP, :],
                            start=True, stop=False,
                        )
                        nc.tensor.matmul(
                            pn[:],
                            s2[:, b * h + mc * 128:b * h + mc * 128 + M[mc]],
                            i8_t[:],
                            start=False, stop=True,
                        )
                        nc.vector.tensor_copy(node[mc][:, b * BPP:(b + 1) * BPP, :], pn[:])

                # u = relu(w1.T @ node + b1)
                for b in range(NBLK):
                    for mc in range(mcs):
                        pu = ps.tile([M[mc], BPP, N], FP, name=f"pu_{l}_{b}_{mc}", tag="pmm")
                        for kc in range(kcs):
                            ksz = min(128, h - kc * 128)
                            nc.tensor.matmul(
                                pu[:],
                                pk[0:ksz, co["w1"] + kc * h + mc * 128:co["w1"] + kc * h + mc * 128 + M[mc]],
                                node[kc][:, b * BPP:(b + 1) * BPP, :],
                                start=(kc == 0), stop=(kc == kcs - 1),
                            )
                        nc.scalar.activation(
                            u[mc][:, b * BPP:(b + 1) * BPP, :], pu[:], AF.Relu,
                            bias=pk[0:M[mc], co["b1"] + mc:co["b1"] + mc + 1].bitcast(FP),
                        )

                # xn = w2.T @ u + b2
                for b in range(NBLK):
                    for mc in range(mcs):
                        px = ps.tile([M[mc], BPP, N], FP, name=f"px_{l}_{b}_{mc}", tag="pmm")
                        for kc in range(kcs):
                            ksz = min(128, h - kc * 128)
                            nc.tensor.matmul(
                                px[:],
                                pk[0:ksz, co["w2"] + kc * h + mc * 128:co["w2"] + kc * h + mc * 128 + M[mc]],
                                u[kc][:, b * BPP:(b + 1) * BPP, :],
                                start=(kc == 0), stop=(kc == kcs - 1),
                            )
                        nc.vector.tensor_scalar_add(
                            xn[mc][:, b * BPP:(b + 1) * BPP, :], px[:],
                            pk[0:M[mc], co["b2"] + mc:co["b2"] + mc + 1].bitcast(FP),
                        )
                x_cur = xn

            # final masked mean pool: out[mc] = sum_n (x * mi)
            for mc in range(2):
                xm = sb.tile([128, PPC, N], FP, name=f"xm_{mc}", tag="xm")
                nc.vector.tensor_tensor(xm[:], x_cur[mc][:].bitcast(FP), mi_t[:], OP.mult)
                Hf = sb.tile([128, PPC], FP, name=f"Hf_{mc}")
                nc.vector.tensor_reduce(Hf[:], xm[:], AX.X, OP.add)
                nc.sync.dma_start(out_d[mc], Hf[:])

    nc.finalize()
    return nc


_OFFS = None


def run(x, masks, params, trace=False):
    global _OFFS
    x = np.asarray(x, dtype=np.float32)
    m = np.asarray(masks)
    pars = [[np.asarray(t, dtype=np.float32) for t in layer] for layer in params]

    packs, offs = _build_packs(pars)
    _OFFS = offs

    i8 = np.zeros((BPP, BPP * N), np.float32)
    for k in range(BPP):
        i8[k, k * N:(k + 1) * N] = 1.0

    xf = x.reshape(B * P, N, F)
    mf = m.reshape(B * P, N).astype(np.float32)

    in_maps = []
    for c in range(NCORES):
        s = slice(c * PPC, (c + 1) * PPC)
        xc = np.ascontiguousarray(xf[s].transpose(2, 0, 1))          # [F, PPC, N]
        mc_ = mf[s]                                                  # [PPC, N]
        cnt = mc_.sum(axis=1, dtype=np.float32)
        inv = np.float32(1.0) / (cnt + EPS)
        alpha = cnt * inv
        mi = np.ascontiguousarray(
            np.broadcast_to((mc_ * inv[:, None])[None], (128, PPC, N))
        )
        al = np.ascontiguousarray(
            np.broadcast_to(alpha[None, :, None], (128, PPC, N))
        )
        im = {
            "xin": xc,
            "mi": mi,
            "al": al,
            "avec": np.ascontiguousarray(alpha.reshape(1, PPC)),
            "i8": i8,
        }
        for l in range(L):
            im[f"pk{l}"] = packs[l]
        in_maps.append(im)

    nc = _build_program([p.shape[1] for p in packs])
    res = run_bass_kernel_spmd(nc, in_maps, list(range(NCORES)), trace=trace)

    outs = []
    for c in range(NCORES):
        o = res.results[c]["out"].reshape(2 * 128, PPC).T             # [PPC, 256]
        outs.append(o)
    full = np.concatenate(outs, axis=0).reshape(B, P, 2 * 128)
    return np.ascontiguousarray(full.astype(np.float32)), res.exec_time_ns


def kernel(**inputs):
    out, _ = run(**inputs)
    return out
